# revision 75
# baseline (speedup 1.0000x reference)
"""Trainium2 Bass kernel for a ViT-style transformer block (B=16, N=577,
H=768, NH=12, MLP=3072) with the reference's Q@V^T attention quirk.

Sharding: data-parallel over batch — 8 NeuronCores x 2 batch items each.
All weights replicated. On-chip dataflow is channel-major ("CT": channels
on partitions, tokens on the free axis); the host pre-transposes x and
back-transposes the output.

Precision split: the attention path runs fp8e4m3 with DoubleRow matmuls
(2 k-tiles per instruction) — q/v/v_tm projections from an fp8 copy of x
with host-scaled fp8 wq/wv, exp probs (E) and token-major values (v_tm)
in fp8 feeding DoubleRow pnt, fp8 oT into a DoubleRow wo. Errors there
average out across 577 tokens before reaching the residual-dominated
output (measured ~1e-3 contribution). fc1/fc2 stay bf16: their fp8
error (~1.2e-2) does not average and would blow the 2e-2 budget.

LN1 never materializes a normalized activation: q/v/v_tm accumulate
Wx directly from x, an extra K=128 matmul adds the (-mu*colsum + std*b)
correction (mwide rows x host cqv consts), and the per-token 1/std lands
as one PSUM-sourced multiply per tile. So the projections depend only on
the x DMA, not the LN latency chain.

Schedule highlights:
 - v_tm fills the PE during the LN1 stats chain; qv(b1) rides inside
   att(b0)'s pair loop, wo(b0)/ln2(b0)/fc1(b0,j<12) inside att(b1)'s;
   wo(b1) interleaves with fc1(b0,j>=12) so the in-order PE stream has
   fill while LN2(b1)'s latency chain runs;
 - scores for a head pair (partitions 0-63/64-127 of one vT/qT tile)
   are emitted adjacently: their K=64 matmuls auto-derive PE row-tiles
   T0/T8 and run concurrently; both denominators broadcast into one
   PSUM tile via masked-ones lhsT rows;
 - deferred gelu: mid-attention fc1 tiles write raw+bias on DVE and the
   gelu is applied in place later behind a data gate, so the Exp ACT
   table is never thrashed inside the attention window.

Token axis padded to 580 (2x290 PSUM halves); fp8 k-tile strides padded
to 16B multiples (TEP/VTP/TUP) for DoubleRow APs.

Self-contained: hardcodes shapes; only needs /opt/trn_rl_repo.
"""
import sys

sys.path.insert(0, "/opt/trn_rl_repo")

import numpy as np
import ml_dtypes

import concourse.bass as bass
import concourse.tile as tile
from concourse import bacc, mybir
from concourse import bass_utils

P = 128
B, T, H, NH, HD, MLP = 16, 577, 768, 12, 64, 3072
NCORES = 8
B2 = B // NCORES          # batch items per core
KT = H // P               # 6 hidden tiles
JT = MLP // P             # 24 mlp tiles
TP = 640                  # padded token columns for lhsT-windowed CT tensors
TE = 580                  # padded token columns (2 x 290, even halves)
HB = 290                  # token half-block
TT = [(0, 128), (128, 128), (256, 128), (384, 128), (512, 65)]  # token tiles
EPS = 1e-6

f32 = mybir.dt.float32
f32r = mybir.dt.float32r
bf16 = mybir.dt.bfloat16
fp8 = mybir.dt.float8e4
AF = mybir.ActivationFunctionType
OP = mybir.AluOpType
DR = mybir.MatmulPerfMode.DoubleRow
S1 = 64.0   # host scales w1 by S1 (fp8 range); epilogue multiplies by 1/S1
S2 = 64.0   # same for w2
SA = 16.0   # host scale for fp8 wq/wv/wo (attention path)
FP8_FC1 = False  # fc1 in fp8 DoubleRow (y2, w1) — too lossy, keep off
FP8_FC2 = False  # fc2 in fp8 DoubleRow (mtbox, w2) — too lossy, keep off
TEP = 592        # fp8 k-tile-padded token columns (DoubleRow needs 16B strides)
VTP = 784        # padded NH*65 for v_tm fp8 DoubleRow
TUP = 1168       # padded B2*TE for oT fp8 DoubleRow


def halves(pt, parts=P):
    """View a [*,1024] two-bank psum tile as [parts, 2, HB] (cols 0.. and 512..)."""
    return pt.rearrange("p (two c) -> p two c", two=2)[:parts, :, :HB]


def build_program(repeat=1, upto="fc2"):
    nc = bacc.Bacc("TRN2", target_bir_lowering=False, debug=False, num_devices=NCORES)

    xt_d = nc.dram_tensor("xt", [B2, H, T], bf16, kind="ExternalInput").ap()
    xt8_d = nc.dram_tensor("xt8", [B2, H, T], fp8, kind="ExternalInput").ap()
    wq_d = nc.dram_tensor("wq", [KT, P, KT, P], fp8, kind="ExternalInput").ap()
    wv_d = nc.dram_tensor("wv", [H, H], fp8, kind="ExternalInput").ap()
    cqv_d = nc.dram_tensor("cqv", [P, 3, KT, P], bf16, kind="ExternalInput").ap()
    wo_d = nc.dram_tensor("wo", [KT, P, KT, P], fp8, kind="ExternalInput").ap()
    w1_d = nc.dram_tensor("w1", [JT, P, KT, P], fp8 if FP8_FC1 else bf16,
                          kind="ExternalInput").ap()
    w2_d = nc.dram_tensor("w2", [KT, P, JT, P], fp8 if FP8_FC2 else bf16,
                          kind="ExternalInput").ap()
    bo_d = nc.dram_tensor("bo", [H], f32, kind="ExternalInput").ap()
    b1_d = nc.dram_tensor("b1", [MLP], f32, kind="ExternalInput").ap()
    b2_d = nc.dram_tensor("b2", [H], f32, kind="ExternalInput").ap()
    out_d = nc.dram_tensor("outt", [B2, H, T], f32, kind="ExternalOutput").ap()

    with tile.TileContext(nc) as tc:
        with (
            tc.tile_pool(name="persist", bufs=1) as persist,
            tc.tile_pool(name="wstream", bufs=4) as wstream,
            tc.tile_pool(name="w2pool", bufs=2) as w2pool,
            tc.tile_pool(name="epool", bufs=5, space="SBUF") as epool,
            tc.tile_pool(name="scratch", bufs=2) as scratch,
            tc.tile_pool(name="rbpool", bufs=2) as rbpool,
            tc.tile_pool(name="rows", bufs=1) as rows,
            tc.tile_pool(name="outp", bufs=2) as outp,
            tc.tile_pool(name="consts", bufs=1) as consts,
        ):
            ones_k = consts.tile([P, 1], f32r, tag="ones_k")
            nc.vector.memset(ones_k.bitcast(f32), 1.0)
            ones_kb = consts.tile([P, 1], bf16, tag="ones_kb")
            nc.vector.memset(ones_kb, 1.0)
            ones_r = consts.tile([1, P], f32r, tag="ones_r")
            nc.vector.memset(ones_r.bitcast(f32), 1.0)
            # masked ones rows: mask10 = 1 on cols 0-63, mask01 = 1 on 64-127;
            # used to land two heads' denominator broadcasts in one PSUM tile
            mask10 = consts.tile([1, P], f32r, tag="mask10")
            nc.vector.memset(mask10.bitcast(f32), 0.0)
            nc.vector.memset(mask10.bitcast(f32)[:, :HD], 1.0)
            mask01 = consts.tile([1, P], f32r, tag="mask01")
            nc.vector.memset(mask01.bitcast(f32), 0.0)
            nc.vector.memset(mask01.bitcast(f32)[:, HD:], 1.0)
            # the std broadcast is pre-scaled by SA so rb_r = 1/(SA*std)
            # also undoes the fp8 weight scaling in the q/v epilogues
            ones_rb = consts.tile([1, P], bf16, tag="ones_rb")
            nc.vector.memset(ones_rb, SA)
            eps_t = consts.tile([1, 1], f32, tag="eps_t")
            nc.vector.memset(eps_t, EPS)
            id2 = consts.tile([1, 1], f32, tag="id2")
            nc.vector.memset(id2, 1.0)
            id2b = consts.tile([1, 1], bf16, tag="id2b")
            nc.vector.memset(id2b, 1.0)

            const_dmas = []

            def col_load(name, dram, ntiles):
                t = consts.tile([P, ntiles], f32, tag=name)
                # DMA issued inside emit_block, after wv/ncs on the ACT queue
                const_dmas.append((t, dram.rearrange("(k p) -> p k", p=P)))
                return t

            bo_c = col_load("bo_c", bo_d, KT)
            b2_c = col_load("b2_c", b2_d, KT)
            b1_c = col_load("b1_c", b1_d, JT)

            for _ in range(repeat):
                emit_block(nc, tc, persist, wstream, w2pool, epool, scratch, rbpool,
                           rows, outp, ones_k, ones_kb, ones_r, ones_rb, eps_t, id2,
                           id2b, mask10, mask01, const_dmas, bo_c, b2_c, b1_c,
                           xt_d, xt8_d, wq_d, wv_d, cqv_d, wo_d, w1_d, w2_d, out_d,
                           upto)
    # Steer the ACT table chooser: hide Exp from exp_and_others (set ids /
    # positions preserved) so every Exp/Ln lands in natural_log_exp_and_others
    # and the kernel needs just one non-gelu table set. Patch is transient,
    # restored right after compile.
    _orig_gat = bacc.get_activation_tables

    def _filtered_gat(arch):
        t = dict(_orig_gat(arch))
        for k in list(t):
            if k not in ("natural_log_exp_and_others", "gelu_and_others"):
                t[k] = set()
        return t

    bacc.get_activation_tables = _filtered_gat
    try:
        nc.compile()
    finally:
        bacc.get_activation_tables = _orig_gat
    return nc


def emit_block(nc, tc, persist, wstream, w2pool, epool, scratch, rbpool, rows, outp,
               ones_k, ones_kb, ones_r, ones_rb, eps_t, id2, id2b, mask10, mask01,
               const_dmas, bo_c, b2_c, b1_c,
               xt_d, xt8_d, wq_d, wv_d, cqv_d, wo_d, w1_d, w2_d, out_d, upto="fc2"):
    TU = B2 * TE
    cu = lambda b: b * TE
    ct = lambda b: b * TP

    # Persistent tensors. Tags encode slot reuse; each second tenant's first
    # write is ordered after the first tenant's last read on every engine.
    xTs = [persist.tile([P, KT, TE], bf16, tag="sC", name="xT0"),
           persist.tile([P, KT, TE], bf16, tag="sD", name="xT1")]
    xT8s = [persist.tile([P, KT, TEP], fp8, tag="sX0", name="xT8_0"),
            persist.tile([P, KT, TEP], fp8, tag="sX1", name="xT8_1")]
    qT = persist.tile([P, KT, TU], bf16, tag="sE")
    vT = persist.tile([P, KT, B2 * TP], bf16, tag="sF")
    v_tm = persist.tile([P, B2, 5, VTP], fp8, tag="sG")
    wv_sb = persist.tile([P, KT, H], fp8, tag="sB")
    cqv_sb = persist.tile([P, 3, KT, P], bf16, tag="sQ")
    # mwide[b]: rows 0 = -mu, 1 = std, 2.. = 0 — the K=128 LN-correction
    # rhs folded into the q/v accumulations (q = rb_r * (Wq^T x - mu*csum + std*bq))
    mwides = [persist.tile([P, TE], bf16, tag="sM0", name="mw0"),
              persist.tile([P, TE], bf16, tag="sM1", name="mw1")]
    rtms = [rows.tile([P, 5, 2], f32, tag="rtm0", name="rtm0"),
            rows.tile([P, 5, 2], f32, tag="rtm1", name="rtm1")]
    rb_r_sbs = [None, None]

    def f(ap):
        return ap.bitcast(f32)

    def two(ap):
        return ap.rearrange("p (two c) -> p two c", two=2)

    # ---- load x (zero the 3 pad token columns) and wv ----
    for b in range(B2):
        for kt in range(KT):
            nc.sync.dma_start(xTs[b][:, kt, :T], xt_d[b, kt * P:(kt + 1) * P, :])
            nc.gpsimd.memset(xTs[b][:, kt, T:TE], 0.0)
            nc.sync.dma_start(xT8s[b][:, kt, :T], xt8_d[b, kt * P:(kt + 1) * P, :])
            nc.gpsimd.memset(xT8s[b][:, kt, T:TE], 0.0)
    nc.scalar.dma_start(wv_sb, wv_d.rearrange("(kt p) n -> p kt n", p=P))
    nc.scalar.dma_start(cqv_sb, cqv_d)
    for _t, _ap in const_dmas:
        nc.scalar.dma_start(_t, _ap)
    const_dmas.clear()
    for b in range(B2):
        for kt in range(KT):
            nc.gpsimd.memset(vT[:, kt, ct(b) + TE:ct(b) + TP], 0.0)

    # ---- LN1: stats only (no apply pass). Produces, per item:
    #  mwides[b] rows (-mu, std) for the in-psum correction matmul,
    #  rb_r_sbs[b] = broadcast 1/std for the q/v epilogue multiply,
    #  rtms[b] r-columns (token-major rstd) for the v_tm epilogue. ----
    def ln1_stats(b, ps):
        src = xTs[b]
        s1 = ps.tile([1, 1024], f32, tag="p", name="ln_s1")
        for kt in range(KT):
            for i in range(2):
                nc.tensor.matmul(s1[:, i * 512:i * 512 + HB], ones_kb,
                                 src[:, kt, i * HB:(i + 1) * HB],
                                 start=kt == 0, stop=kt == KT - 1)
        s2 = ps.tile([1, 1024], f32, tag="p", name="ln_s2")
        for kt in range(KT):
            sq = scratch.tile([P, TE], bf16, tag="sq")
            nc.vector.tensor_tensor(sq, src[:, kt, :], src[:, kt, :], OP.mult)
            for i in range(2):
                nc.tensor.matmul(s2[:, i * 512:i * 512 + HB], ones_kb,
                                 sq[:, i * HB:(i + 1) * HB],
                                 start=kt == 0, stop=kt == KT - 1)
        mu = rows.tile([1, TE], f32, tag="mu")
        nc.vector.tensor_scalar_mul(two(mu), halves(s1, 1), 1.0 / H)
        m2 = rows.tile([1, TE], f32, tag="var")
        nc.vector.tensor_tensor(m2, mu, mu, OP.mult)
        std = rows.tile([1, TE], f32, tag="std")
        nc.vector.tensor_scalar(two(std), halves(s2, 1), 1.0 / H, EPS,
                                OP.mult, OP.add)
        nc.vector.tensor_tensor(std, std, m2, OP.subtract)
        # sqrt via exp(0.5*ln(v)): Ln+Exp share one ACT table set with the
        # attention exps, so no sqrt_and_others table switches anywhere
        nc.scalar.activation(std, std, AF.Ln)
        nc.scalar.activation(std, std, AF.Exp, scale=0.5)
        nc.vector.tensor_copy(mwides[b][0:1, :], std)
        # -mu lives on partition 32: engine APs may only start at 0/32/64/96
        nc.vector.tensor_scalar_mul(mwides[b][32:33, :], mu, -1.0)
        rbp = ps.tile([P, 1024], f32, tag="p", name="ln_rbp")
        for i in range(2):
            nc.tensor.matmul(rbp[:, i * 512:i * 512 + HB], ones_rb,
                             mwides[b][0:1, i * HB:(i + 1) * HB],
                             start=True, stop=True)
        rb_r_sb = rbpool.tile([P, TE], bf16, tag="rbr", name="rb_r_sb")
        with nc.allow_low_precision(reason="bf16 rounding of rstd broadcast"):
            nc.vector.reciprocal(two(rb_r_sb), halves(rbp))
        rb_r_sbs[b] = rb_r_sb
        for ti in range(5):
            t0, tl = TT[ti]
            w = min(TE - t0, P)
            ptr = ps.tile([P, 1024], f32, tag="p", name="tr_ps")
            nc.tensor.transpose(ptr.bitcast(bf16)[:w, 0:1],
                                mwides[b][0:1, t0:t0 + w], id2b)
            with nc.allow_low_precision(reason="f32 recip of std column"):
                nc.vector.reciprocal(rtms[b][:w, ti, 0:1],
                                     ptr.bitcast(bf16)[:w, 0:1])

    # ---- layernorm with apply (used for LN2 only) ----
    def layernorm_b(src, s0, dst, d0, ps):
        bfsrc = src.dtype == bf16
        sf = (lambda ap: ap) if bfsrc else f
        ones_s1 = ones_kb if bfsrc else ones_k
        sfull = lambda kt: src[:, kt, s0:s0 + TE]
        shalf = lambda kt, i: src[:, kt, s0 + i * HB:s0 + (i + 1) * HB]
        s1 = ps.tile([1, 1024], f32, tag="p", name="ln_s1")
        for kt in range(KT):
            for i in range(2):
                nc.tensor.matmul(s1[:, i * 512:i * 512 + HB], ones_s1, shalf(kt, i),
                                 start=kt == 0, stop=kt == KT - 1)
        s2 = ps.tile([1, 1024], f32, tag="p", name="ln_s2")
        for kt in range(KT):
            sq = scratch.tile([P, TE], bf16, tag="sq")
            nc.vector.tensor_tensor(sq, sf(sfull(kt)), sf(sfull(kt)), OP.mult)
            for i in range(2):
                nc.tensor.matmul(s2[:, i * 512:i * 512 + HB], ones_kb,
                                 sq[:, i * HB:(i + 1) * HB],
                                 start=kt == 0, stop=kt == KT - 1)
        mu = rows.tile([1, TE], f32, tag="mu")
        nc.vector.tensor_scalar_mul(two(mu), halves(s1, 1), 1.0 / H)
        var = rows.tile([1, TE], f32, tag="var")
        nc.vector.tensor_tensor(var, mu, mu, OP.mult)
        nc.vector.scalar_tensor_tensor(two(var), halves(s2, 1), 1.0 / H,
                                       two(var), OP.mult, OP.subtract)
        nc.scalar.activation(var, var, AF.Ln, bias=eps_t)
        nc.scalar.activation(var, var, AF.Exp, scale=0.5)
        rstd = rows.tile([1, TE], f32r, tag="rstd")
        with nc.allow_low_precision(reason="f32r rounding of rstd"):
            nc.vector.reciprocal(rstd, var)
        murstd = rows.tile([1, TE], f32r, tag="var")
        nc.vector.tensor_tensor(murstd, mu, f(rstd), OP.mult)
        rb_r = ps.tile([P, 1024], f32, tag="p", name="ln_rbr")
        rb_mr = ps.tile([P, 1024], f32, tag="p", name="ln_rbmr")
        for i in range(2):
            nc.tensor.matmul(rb_r[:, i * 512:i * 512 + HB], ones_r,
                             rstd[:, i * HB:(i + 1) * HB], start=True, stop=True)
            nc.tensor.matmul(rb_mr[:, i * 512:i * 512 + HB], ones_r,
                             murstd[:, i * HB:(i + 1) * HB], start=True, stop=True)
        rb_r_sb = rbpool.tile([P, TE], bf16, tag="rbr", name="rb_r_sb")
        nc.vector.tensor_copy(two(rb_r_sb), halves(rb_r))
        rb_mr_sb = rbpool.tile([P, TE], bf16, tag="rbmr", name="rb_mr_sb")
        nc.scalar.activation(two(rb_mr_sb), halves(rb_mr), AF.Identity)
        for kt in range(KT):
            tmp = scratch.tile([P, TE], bf16, tag="lntmp")
            nc.vector.tensor_tensor(tmp, sf(sfull(kt)), rb_r_sb, OP.mult)
            nc.vector.tensor_tensor(dst[:, kt, d0:d0 + TE], tmp,
                                    rb_mr_sb, OP.subtract)
        return rstd, murstd

    with (tc.tile_pool(name="ps_ln1", bufs=2, space="PSUM") as ps_ln,
          tc.tile_pool(name="ps_vtm", bufs=4, space="PSUM") as ps_vtm):
        # token-major v from xT directly (no LN dependency): the matmuls fill
        # the PE during the LN1 latency chain; the -mu*colsum(wv) correction
        # rides as an extra K=128 matmul (mwide rows x cqv slot 2), so the
        # epilogue is a single per-token r multiply:
        #   v_tm[t,c] = r[t]*((Wv^T x)[t,c] - mu[t]*colsum(wv)[c])
        vtm_ps = {}

        def vtm_mms(b, ti, nb, pool=None, ptag="pv", psz=512):
            t0, tl = TT[ti]
            w = min(TE - t0, P)
            pv = (pool or ps_vtm).tile([P, psz], f32, tag=ptag, name="vtm_ps")
            for kp in range(KT // 2):
                nc.tensor.matmul(
                    pv[:w, :384], xT8s[b][:, 2 * kp:2 * kp + 2, t0:t0 + w],
                    wv_sb[:, 2 * kp:2 * kp + 2, nb * 384:(nb + 1) * 384],
                    start=kp == 0, stop=False, perf_mode=DR)
            nc.tensor.matmul(
                pv[:w, :384], mwides[b][:, t0:t0 + w],
                cqv_sb[:, 2].rearrange("p kt n -> p (kt n)")[:, nb * 384:(nb + 1) * 384],
                start=False, stop=True)
            vtm_ps[(b, ti, nb)] = pv

        def vtm_epi(b, ti, nb):
            rr = 128 if ti < 4 else 65
            pv = vtm_ps.pop((b, ti, nb))
            r_tm = rtms[b][:rr, ti, 0:1]
            nc.vector.tensor_scalar(
                v_tm[:rr, b, ti, 65 * nb * 6: 65 * (nb + 1) * 6]
                    .rearrange("p (h d) -> p h d", d=65)[:, :, :HD],
                pv[:rr, :384].rearrange("p (h d) -> p h d", d=HD),
                r_tm, 1.0 / SA, OP.mult, OP.mult)

        for b in range(B2):
            nc.gpsimd.memset(mwides[b], 0.0)
            nc.gpsimd.memset(v_tm[:, b, 4, :], 0.0)
            for ti in range(5):
                rr = 128 if ti < 4 else 65
                nc.gpsimd.memset(v_tm[:rr, b, ti, HD::65], 1.0)
        ln1_stats(0, ps_ln)
        for ti in range(5):
            for nb in range(2):
                vtm_mms(0, ti, nb)
        ln1_stats(1, ps_ln)
        for ti in range(5):
            for nb in range(2):
                vtm_mms(1, ti, nb)
        for b in range(B2):
            for ti in range(5):
                for nb in range(2):
                    vtm_epi(b, ti, nb)
    if upto == "ln1":
        return

    # ---- q/v, attention, wo+LN2, MLP — emitted with cross-phase interleave:
    # qv(b1) units ride inside att(b0)'s head loop, fc1(b0, j0-11) units ride
    # inside att(b1)'s. Emission order drives both the scheduler priorities
    # and the PSUM tag-"p" slot-reuse chain, so PE fill work is available
    # whenever the softmax (ACT) pipeline lags. ----
    oT = persist.tile([P, KT, TUP], fp8, tag="sH")
    x1T = persist.tile([P, KT, TU], bf16, tag="sA")
    y2dt = fp8 if FP8_FC1 else bf16
    y2Ts = [persist.tile([P, KT, TE], y2dt, tag="sB", name="y2T0"),
            persist.tile([P, KT, TE], y2dt, tag="sC", name="y2T1")]
    mtbox = [None]  # fc1 output chunks; 2 fresh slots + 2 reused (sG/sH)

    with (tc.tile_pool(name="ps_mid", bufs=2, space="PSUM") as ps,
          tc.tile_pool(name="ps_sc", bufs=2, space="PSUM") as ps_sc):

        def q_unit(b, j):
            wqj = wstream.tile([P, KT, P], fp8, tag="w8", name="wq_j")
            nc.sync.dma_start(wqj, wq_d[j])
            pq = ps.tile([P, 1024], f32, tag="p", name="q_ps")
            for kp in range(KT // 2):
                for i in range(2):
                    nc.tensor.matmul(
                        pq[:, i * 512:i * 512 + HB], wqj[:, 2 * kp:2 * kp + 2, :],
                        xT8s[b][:, 2 * kp:2 * kp + 2, i * HB:(i + 1) * HB],
                        start=kp == 0, stop=False, perf_mode=DR)
            for i in range(2):
                nc.tensor.matmul(pq[:, i * 512:i * 512 + HB], cqv_sb[:, 0, j, :],
                                 mwides[b][:, i * HB:(i + 1) * HB],
                                 start=False, stop=True)
            nc.vector.tensor_tensor(two(qT[:, j, cu(b):cu(b) + TE]),
                                    halves(pq), two(rb_r_sbs[b]), OP.mult)

        def v_unit(b, j):
            pv = ps.tile([P, 1024], f32, tag="p", name="v_ps")
            for kp in range(KT // 2):
                for i in range(2):
                    nc.tensor.matmul(
                        pv[:, i * 512:i * 512 + HB],
                        wv_sb[:, 2 * kp:2 * kp + 2, j * P:(j + 1) * P],
                        xT8s[b][:, 2 * kp:2 * kp + 2, i * HB:(i + 1) * HB],
                        start=kp == 0, stop=False, perf_mode=DR)
            for i in range(2):
                nc.tensor.matmul(pv[:, i * 512:i * 512 + HB], cqv_sb[:, 1, j, :],
                                 mwides[b][:, i * HB:(i + 1) * HB],
                                 start=False, stop=True)
            nc.vector.tensor_tensor(two(vT[:, j, ct(b):ct(b) + TE]),
                                    halves(pv), two(rb_r_sbs[b]), OP.mult)

        def qv_units(b):
            us = [lambda j=j: q_unit(b, j) for j in range(KT)]
            us += [lambda j=j: v_unit(b, j) for j in range(KT)]
            return us

        def scores_pair(b, hp):
            # heads (2hp, 2hp+1) live on partitions 0-63 / 64-127 of the same
            # vT/qT tile, so their K=64 matmuls land on PE row-tiles T0/T8.
            # Emitting them adjacently lets the halves run concurrently.
            jh = hp
            E0 = epool.tile([P, 5, TEP], fp8, tag="E", name="E0")
            E1 = epool.tile([P, 5, TEP], fp8, tag="E", name="E1")
            for kt in range(5):
                ps0 = ps_sc.tile([P, 1024], f32, tag="pss", name="s_ps0")
                ps1 = ps_sc.tile([P, 1024], f32, tag="pss", name="s_ps1")
                for i in range(2):
                    for ph, pst in ((0, ps0), (HD, ps1)):
                        nc.tensor.matmul(
                            pst[:, i * 512:i * 512 + HB],
                            vT[ph:ph + HD, jh, ct(b) + kt * P: ct(b) + (kt + 1) * P],
                            qT[ph:ph + HD, jh, cu(b) + i * HB: cu(b) + (i + 1) * HB],
                            start=True, stop=True)
                nc.scalar.activation(two(E0[:, kt, :TE]), halves(ps0),
                                     AF.Exp, scale=0.125)
                nc.scalar.activation(two(E1[:, kt, :TE]), halves(ps1),
                                     AF.Exp, scale=0.125)
            return E0, E1

        def out_pair(b, h0, E0, E1):
            # pnt for both heads, then the reciprocal broadcasts packed into
            # one PSUM tile on column-tiles (0,0)/(0,64) so they overlap.
            pnts = []
            for h, E in ((h0, E0), (h0 + 1, E1)):
                pnt = ps.tile([P, 1024], f32, tag="p", name="nt_ps")
                for i in range(2):
                    for m in range(2):
                        nc.tensor.matmul(pnt[:65, i * 512:i * 512 + HB],
                                         v_tm[:, b, 2 * m:2 * m + 2, 65 * h: 65 * h + 65],
                                         E[:, 2 * m:2 * m + 2, i * HB:(i + 1) * HB],
                                         start=m == 0, stop=False, perf_mode=DR)
                    nc.tensor.matmul(pnt[:65, i * 512:i * 512 + HB],
                                     v_tm[:, b, 4, 65 * h: 65 * h + 65],
                                     E[:, 4, i * HB:(i + 1) * HB],
                                     start=False, stop=True)
                pnts.append(pnt)
            rs = []
            for k, pnt in enumerate(pnts):
                r = rows.tile([1, TE], f32r, tag=f"r{k}", name=f"r{k}")
                with nc.allow_low_precision(reason="f32r rounding of softmax denom"):
                    nc.vector.reciprocal(two(r), halves(pnt, 128)[64:65])
                rs.append(r)
            prr = ps_sc.tile([P, 1024], f32, tag="pss", name="rep_ps")
            for k, (r, msk) in enumerate(zip(rs, (mask10, mask01))):
                for i in range(2):
                    nc.tensor.matmul(prr[:, i * 512:i * 512 + HB], msk,
                                     r[:, i * HB:(i + 1) * HB],
                                     start=k == 0, stop=k == 1)
            prr2 = prr.rearrange("p (two c) -> p two c", two=2)
            for k, h in enumerate((h0, h0 + 1)):
                jh, ph = h // 2, HD * (h % 2)
                pnt = pnts[k]
                ntsb = scratch.tile([P, TE], f32, tag="ntsb")
                if k == 0:
                    nc.vector.tensor_copy(two(ntsb[:HD]), halves(pnt, HD))
                else:
                    # split the PSUM evacuations across DVE and ACT (Copy is
                    # in every ACT table set, so no table-switch cost)
                    nc.scalar.activation(two(ntsb[:HD]), halves(pnt, HD),
                                         AF.Copy)
                nc.vector.tensor_tensor(
                    two(oT[ph:ph + HD, jh, cu(b):cu(b) + TE]),
                    two(ntsb[:HD]), prr2[64 * k:64 * k + HD, :, :HB], OP.mult)

        def attention_b(b, fill, per_head):
            # scores run one pair ahead of out_pair so the exp (ACT) chain
            # never waits behind pnt/prr matmuls in the PE queue
            Es = {}
            NP = NH // 2
            for hp in range(NP):
                Es[hp] = scores_pair(b, hp)
                if hp >= 1:
                    E0, E1 = Es.pop(hp - 1)
                    out_pair(b, 2 * (hp - 1), E0, E1)
                for _ in range(2 * per_head):
                    if fill:
                        fill.pop(0)()
            E0, E1 = Es.pop(NP - 1)
            out_pair(b, 2 * (NP - 1), E0, E1)
            while fill:
                fill.pop(0)()

        def wo_unit(b, j):
            woj = wstream.tile([P, KT, P], fp8, tag="w8", name="wo_j")
            nc.sync.dma_start(woj, wo_d[j])
            po = ps.tile([P, 1024], f32, tag="p", name="wo_ps")
            for kp in range(KT // 2):
                for i in range(2):
                    nc.tensor.matmul(
                        po[:, i * 512:i * 512 + HB], woj[:, 2 * kp:2 * kp + 2, :],
                        oT[:, 2 * kp:2 * kp + 2, cu(b) + i * HB: cu(b) + (i + 1) * HB],
                        start=kp == 0, stop=kp == KT // 2 - 1, perf_mode=DR)
            # (po/SA + bo) on DVE, + residual on Pool (idle mid-attention)
            x1w = scratch.tile([P, TE], f32, tag="x1w")
            nc.vector.tensor_scalar(two(x1w), halves(po), 1.0 / SA,
                                    bo_c[:, j:j + 1], OP.mult, OP.add)
            nc.gpsimd.tensor_tensor(x1T[:, j, cu(b):cu(b) + TE], x1w,
                                    xTs[b][:, j, :], OP.add)

        def wo_ln2(b):
            for j in range(KT):
                wo_unit(b, j)
            layernorm_b(x1T, cu(b), y2Ts[b], 0, ps)

        def fc1_unit(bb, j, defer_gelu=False, pool=None, ptag="p"):
            w1j = wstream.tile([P, KT, P], fp8 if FP8_FC1 else bf16,
                               tag="w", name="w1_j")
            nc.sync.dma_start(w1j, w1_d[j])
            pm = (pool or ps).tile([P, 1024], f32, tag=ptag, name="fc1_ps")
            if FP8_FC1:
                for kp in range(KT // 2):
                    for i in range(2):
                        nc.tensor.matmul(
                            pm[:, i * 512:i * 512 + HB], w1j[:, 2 * kp:2 * kp + 2, :],
                            y2Ts[bb][:, 2 * kp:2 * kp + 2, i * HB:(i + 1) * HB],
                            start=kp == 0, stop=kp == KT // 2 - 1, perf_mode=DR)
            else:
                for kt in range(KT):
                    for i in range(2):
                        nc.tensor.matmul(
                            pm[:, i * 512:i * 512 + HB], w1j[:, kt, :],
                            y2Ts[bb][:, kt, i * HB:(i + 1) * HB],
                            start=kt == 0, stop=kt == KT - 1)
            sc1 = (1.0 / S1) if FP8_FC1 else 1.0
            dst = two(mtbox[0][j // KT][:, j % KT, cu(bb):cu(bb) + TE])
            if defer_gelu:
                # raw+bias on DVE (ACT is exp-saturated mid-attention);
                # gelu applied in place later, gated behind attention's end
                nc.vector.tensor_scalar(dst, halves(pm), sc1,
                                        b1_c[:, j:j + 1], OP.mult, OP.add)
            else:
                nc.scalar.activation(dst, halves(pm), AF.Gelu,
                                     bias=b1_c[:, j:j + 1], scale=sc1)

        def fc2_chain(bb, j2, w2j):
            pf = (ps_sc if (j2 + bb) % 2 else ps).tile(
                [P, 1024], f32, tag="pss" if (j2 + bb) % 2 else "p",
                name="fc2_ps")
            if FP8_FC2:
                for jp in range(JT // 2):
                    for i in range(2):
                        nc.tensor.matmul(
                            pf[:, i * 512:i * 512 + HB], w2j[:, 2 * jp:2 * jp + 2, :],
                            mtbox[0][(2 * jp) // KT][:, (2 * jp) % KT:(2 * jp) % KT + 2,
                                                     cu(bb) + i * HB: cu(bb) + (i + 1) * HB],
                            start=jp == 0, stop=jp == JT // 2 - 1, perf_mode=DR)
                ob = outp.tile([P, TE], f32, tag="ob")
                nc.vector.tensor_scalar(two(ob), halves(pf), 1.0 / S2,
                                        b2_c[:, j2:j2 + 1], OP.mult, OP.add)
                nc.gpsimd.tensor_tensor(ob, ob,
                                        x1T[:, j2, cu(bb):cu(bb) + TE], OP.add)
            else:
                for j in range(JT):
                    for i in range(2):
                        nc.tensor.matmul(
                            pf[:, i * 512:i * 512 + HB], w2j[:, j, :],
                            mtbox[0][j // KT][:, j % KT, cu(bb) + i * HB: cu(bb) + (i + 1) * HB],
                            start=j == 0, stop=j == JT - 1)
                ob = outp.tile([P, TE], f32, tag="ob")
                nc.vector.scalar_tensor_tensor(
                    two(ob), halves(pf), b2_c[:, j2:j2 + 1],
                    two(x1T[:, j2, cu(bb):cu(bb) + TE]), OP.add, OP.add)
            nc.sync.dma_start(
                out_d[bb, j2 * P:(j2 + 1) * P, :], ob[:, :T])

        for u in qv_units(0):
            u()
        if upto == "qv":
            return
        attention_b(0, qv_units(1), 2)
        if upto in ("att", "wo", "ln2"):
            wo_ln2(0)
            return
        mtbox[0] = [persist.tile([P, KT, TU], fp8 if FP8_FC2 else bf16,
                                 tag=t, name=f"mt_{t}")
                    for t in ("sI", "sJ", "sG", "sH")]
        fill1 = [lambda j=j: wo_unit(0, j) for j in range(KT)]
        fill1.append(lambda: layernorm_b(x1T, cu(0), y2Ts[0], 0, ps))
        fill1 += [lambda j=j: fc1_unit(0, j, defer_gelu=True)
                  for j in range(2 * KT)]
        attention_b(1, fill1, 2)
        # wo(1) interleaved with the remaining fc1(0) units (also gelu-
        # deferred) so the in-order PE stream has fill while LN2(b1)'s
        # latency chain runs.
        for j in range(KT):
            wo_unit(1, j)
            fc1_unit(0, 2 * KT + 2 * j, defer_gelu=True, pool=ps_sc, ptag="pss")
            fc1_unit(0, 2 * KT + 2 * j + 1, defer_gelu=True, pool=ps_sc,
                     ptag="pss")
        layernorm_b(x1T, cu(1), y2Ts[1], 0, ps)
        # zero bias gate, data-dependent on the last out_head of att(b1):
        # keeps the deferred gelus (a different ACT table) from hoisting into
        # the attention window and thrashing LoadActFuncSet.
        gate0 = rows.tile([P, 1], f32, tag="gate")
        nc.gpsimd.tensor_scalar(gate0, f(oT.bitcast(f32)[:, KT - 1, (cu(1) + TE) // 4 - 1: (cu(1) + TE) // 4]),
                                0.0, None, OP.mult)
        for j in range(JT):
            sl = mtbox[0][j // KT][:, j % KT, cu(0):cu(0) + TE]
            nc.scalar.activation(sl, sl, AF.Gelu, bias=gate0)
        for j in range(JT):
            fc1_unit(1, j, pool=ps_sc, ptag="pss")
        if upto == "fc1":
            return
        for j2 in range(KT):
            w2j = w2pool.tile([P, JT, P], fp8 if FP8_FC2 else bf16,
                              tag="w2", name="w2_j")
            nc.sync.dma_start(w2j, w2_d[j2])
            for bb in range(B2):
                fc2_chain(bb, j2, w2j)


_cached = {}


def get_program(repeat=1):
    if repeat not in _cached:
        _cached[repeat] = build_program(repeat)
    return _cached[repeat]


def make_in_maps(inputs):
    x = np.asarray(inputs["x"], dtype=np.float32)
    xt_all = np.ascontiguousarray(x.transpose(0, 2, 1))  # [B, H, T]
    g1 = np.asarray(inputs["ln1_g"], np.float64)
    be1 = np.asarray(inputs["ln1_b"], np.float64)
    g2 = np.asarray(inputs["ln2_g"], np.float64)
    be2 = np.asarray(inputs["ln2_b"], np.float64)
    Wq = np.asarray(inputs["Wq"], np.float64)
    Wv = np.asarray(inputs["Wv"], np.float64)
    Wo = np.asarray(inputs["Wo"], np.float64)
    W1 = np.asarray(inputs["W1"], np.float64)
    # Fold LN affine into the consuming projections (exact refactoring).
    wq = g1[:, None] * Wq
    bq = be1 @ Wq + np.asarray(inputs["bq"], np.float64)
    wv = g1[:, None] * Wv
    bv = be1 @ Wv + np.asarray(inputs["bv"], np.float64)
    # v_tm carries no bias; probs rows sum to 1 so P@(1 x bv) == bv -> fold into bo.
    bo = np.asarray(inputs["bo"], np.float64) + bv @ Wo
    w1 = g2[:, None] * W1
    b1 = be2 @ W1 + np.asarray(inputs["b1"], np.float64)
    def prep(w, jt, dt):
        # [H_in, J*128] -> [j, p, kt, n] so each per-j DMA is fully contiguous
        kt = w.shape[0] // P
        return np.ascontiguousarray(
            w.reshape(kt, P, jt, P).transpose(2, 1, 0, 3)).astype(dt)

    # fp8 attention-path weights, scaled by SA; column sums for the LN
    # correction are taken over the QUANTIZED weights so the mean term
    # cancels exactly.
    wq8 = (wq * SA).astype(ml_dtypes.float8_e4m3fn)
    wv8 = (wv * SA).astype(ml_dtypes.float8_e4m3fn)
    wo8 = (np.asarray(inputs["Wo"], np.float64) * SA).astype(ml_dtypes.float8_e4m3fn)
    # LN-correction lhsT consts, paired against mwide rows (0: std, 32: -mu):
    # slot 0 = q (SA*bq, csum(wq8)), slot 1 = v (SA*bv, csum(wv8)),
    # slot 2 = v_tm (0, csum(wv8)); all other rows zero.
    cqv = np.zeros((P, 3, KT, P), np.float64)
    cqv[0, 0] = (SA * bq).reshape(KT, P)
    cqv[32, 0] = wq8.astype(np.float64).sum(axis=0).reshape(KT, P)
    cqv[0, 1] = (SA * bv).reshape(KT, P)
    cqv[32, 1] = wv8.astype(np.float64).sum(axis=0).reshape(KT, P)
    cqv[32, 2] = wv8.astype(np.float64).sum(axis=0).reshape(KT, P)

    def prep8(w8, jt):
        kt = w8.shape[0] // P
        return np.ascontiguousarray(
            w8.reshape(kt, P, jt, P).transpose(2, 1, 0, 3))

    com = {
        "wq": prep8(wq8, KT),
        "cqv": cqv.astype(ml_dtypes.bfloat16),
        "wv": wv8,
        "wo": prep8(wo8, KT),
        "w1": prep(w1 * S1, JT, ml_dtypes.float8_e4m3fn) if FP8_FC1
        else prep(w1, JT, ml_dtypes.bfloat16),
        "w2": prep(np.asarray(inputs["W2"], np.float64) * S2, KT,
                   ml_dtypes.float8_e4m3fn) if FP8_FC2
        else prep(np.asarray(inputs["W2"], np.float64), KT, ml_dtypes.bfloat16),
        "bo": bo.astype(np.float32),
        "b1": b1.astype(np.float32),
        "b2": np.asarray(inputs["b2"], np.float32),
    }
    return [dict(com,
                 xt=np.ascontiguousarray(xt_all[i * B2:(i + 1) * B2]).astype(ml_dtypes.bfloat16),
                 xt8=np.ascontiguousarray(xt_all[i * B2:(i + 1) * B2]).astype(ml_dtypes.float8_e4m3fn))
            for i in range(NCORES)]


def kernel(**inputs):
    nc = get_program()
    in_maps = make_in_maps(inputs)
    res = bass_utils.run_bass_kernel_spmd(nc, in_maps, core_ids=list(range(NCORES)))
    out = np.concatenate([res.results[i]["outt"] for i in range(NCORES)], axis=0)
    return np.ascontiguousarray(out.transpose(0, 2, 1)).astype(np.float32)



# revision 82
# speedup vs baseline: 1.0678x; 1.0678x over previous
"""Trainium2 Bass kernel for a ViT-style transformer block (B=16, N=577,
H=768, NH=12, MLP=3072) with the reference's Q@V^T attention quirk.

Sharding: data-parallel over batch — 8 NeuronCores x 2 batch items each.
All weights replicated. On-chip dataflow is channel-major ("CT": channels
on partitions, tokens on the free axis); the host pre-transposes x and
back-transposes the output.

Precision split: the attention path runs fp8e4m3 with DoubleRow matmuls
(2 k-tiles per instruction) — q/v/v_tm projections from an fp8 copy of x
with host-scaled fp8 wq/wv, exp probs (E) and token-major values (v_tm)
in fp8 feeding DoubleRow pnt, fp8 oT into a DoubleRow wo. Errors there
average out across 577 tokens before reaching the residual-dominated
output (measured ~1e-3 contribution). fc1/fc2 stay bf16: their fp8
error (~1.2e-2) does not average and would blow the 2e-2 budget.

LN1 never materializes a normalized activation: q/v/v_tm accumulate
Wx directly from x, an extra K=128 matmul adds the (-mu*colsum + std*b)
correction (mwide rows x host cqv consts), and the per-token 1/std lands
as one PSUM-sourced multiply per tile. So the projections depend only on
the x DMA, not the LN latency chain.

Schedule highlights:
 - v_tm fills the PE during the LN1 stats chain; qv(b1) rides inside
   att(b0)'s pair loop, wo(b0)/ln2(b0)/fc1(b0,j<12) inside att(b1)'s;
   wo(b1) interleaves with fc1(b0,j>=12) so the in-order PE stream has
   fill while LN2(b1)'s latency chain runs;
 - scores for a head pair (partitions 0-63/64-127 of one vT/qT tile)
   are emitted adjacently: their K=64 matmuls auto-derive PE row-tiles
   T0/T8 and run concurrently; both denominators broadcast into one
   PSUM tile via masked-ones lhsT rows;
 - deferred gelu: mid-attention fc1 tiles write raw+bias on DVE and the
   gelu is applied in place later behind a data gate, so the Exp ACT
   table is never thrashed inside the attention window.

Token axis padded to 580 (2x290 PSUM halves); fp8 k-tile strides padded
to 16B multiples (TEP/VTP/TUP) for DoubleRow APs.

Self-contained: hardcodes shapes; only needs /opt/trn_rl_repo.
"""
import sys

sys.path.insert(0, "/opt/trn_rl_repo")

import numpy as np
import ml_dtypes

import concourse.bass as bass
import concourse.tile as tile
from concourse import bacc, mybir
from concourse import bass_utils

P = 128
B, T, H, NH, HD, MLP = 16, 577, 768, 12, 64, 3072
NCORES = 8
B2 = B // NCORES          # batch items per core
KT = H // P               # 6 hidden tiles
JT = MLP // P             # 24 mlp tiles
TP = 640                  # padded token columns for lhsT-windowed CT tensors
TE = 580                  # padded token columns (2 x 290, even halves)
HB = 290                  # token half-block
TT = [(0, 128), (128, 128), (256, 128), (384, 128), (512, 65)]  # token tiles
EPS = 1e-6

f32 = mybir.dt.float32
f32r = mybir.dt.float32r
bf16 = mybir.dt.bfloat16
fp8 = mybir.dt.float8e4
AF = mybir.ActivationFunctionType
OP = mybir.AluOpType
DR = mybir.MatmulPerfMode.DoubleRow
S1 = 64.0   # host scales w1 by S1 (fp8 range); epilogue multiplies by 1/S1
S2 = 64.0   # same for w2
SA = 16.0   # host scale for fp8 wq/wv/wo (attention path)
FP8_FC1 = False  # fc1 in fp8 DoubleRow (y2, w1) — too lossy, keep off
FP8_FC2 = False  # fc2 in fp8 DoubleRow (mtbox, w2) — too lossy, keep off
TEP = 592        # fp8 k-tile-padded token columns (DoubleRow needs 16B strides)
VTP = 784        # padded NH*65 for v_tm fp8 DoubleRow
TUP = 1168       # padded B2*TE for oT fp8 DoubleRow


def halves(pt, parts=P):
    """View a [*,1024] two-bank psum tile as [parts, 2, HB] (cols 0.. and 512..)."""
    return pt.rearrange("p (two c) -> p two c", two=2)[:parts, :, :HB]


def build_program(repeat=1, upto="fc2"):
    nc = bacc.Bacc("TRN2", target_bir_lowering=False, debug=False, num_devices=NCORES)

    xt_d = nc.dram_tensor("xt", [B2, H, T], bf16, kind="ExternalInput").ap()
    xt8_d = nc.dram_tensor("xt8", [B2, H, T], fp8, kind="ExternalInput").ap()
    wq_d = nc.dram_tensor("wq", [KT, P, KT, P], fp8, kind="ExternalInput").ap()
    wv_d = nc.dram_tensor("wv", [H, H], fp8, kind="ExternalInput").ap()
    cqv_d = nc.dram_tensor("cqv", [P, 3, KT, P], bf16, kind="ExternalInput").ap()
    wo_d = nc.dram_tensor("wo", [KT, P, KT, P], fp8, kind="ExternalInput").ap()
    w1_d = nc.dram_tensor("w1", [JT, P, KT, P], fp8 if FP8_FC1 else bf16,
                          kind="ExternalInput").ap()
    w2_d = nc.dram_tensor("w2", [KT, P, JT, P], fp8 if FP8_FC2 else bf16,
                          kind="ExternalInput").ap()
    bo_d = nc.dram_tensor("bo", [H], f32, kind="ExternalInput").ap()
    b1_d = nc.dram_tensor("b1", [MLP], f32, kind="ExternalInput").ap()
    b2_d = nc.dram_tensor("b2", [H], f32, kind="ExternalInput").ap()
    out_d = nc.dram_tensor("outt", [B2, H, T], f32, kind="ExternalOutput").ap()

    with tile.TileContext(nc) as tc:
        with (
            tc.tile_pool(name="persist", bufs=1) as persist,
            tc.tile_pool(name="wstream", bufs=3) as wstream,
            tc.tile_pool(name="w2pool", bufs=2) as w2pool,
            tc.tile_pool(name="epool", bufs=6, space="SBUF") as epool,
            tc.tile_pool(name="scratch", bufs=2) as scratch,
            tc.tile_pool(name="rbpool", bufs=2) as rbpool,
            tc.tile_pool(name="rows", bufs=1) as rows,
            tc.tile_pool(name="outp", bufs=2) as outp,
            tc.tile_pool(name="consts", bufs=1) as consts,
        ):
            ones_k = consts.tile([P, 1], f32r, tag="ones_k")
            nc.vector.memset(ones_k.bitcast(f32), 1.0)
            ones_kb = consts.tile([P, 1], bf16, tag="ones_kb")
            nc.vector.memset(ones_kb, 1.0)
            ones_r = consts.tile([1, P], f32r, tag="ones_r")
            nc.vector.memset(ones_r.bitcast(f32), 1.0)
            # masked ones rows: mask10 = 1 on cols 0-63, mask01 = 1 on 64-127;
            # used to land two heads' denominator broadcasts in one PSUM tile
            mask10 = consts.tile([1, P], f32r, tag="mask10")
            nc.vector.memset(mask10.bitcast(f32), 0.0)
            nc.vector.memset(mask10.bitcast(f32)[:, :HD], 1.0)
            mask01 = consts.tile([1, P], f32r, tag="mask01")
            nc.vector.memset(mask01.bitcast(f32), 0.0)
            nc.vector.memset(mask01.bitcast(f32)[:, HD:], 1.0)
            # the std broadcast is pre-scaled by SA so rb_r = 1/(SA*std)
            # also undoes the fp8 weight scaling in the q/v epilogues
            ones_rb = consts.tile([1, P], bf16, tag="ones_rb")
            nc.vector.memset(ones_rb, SA)
            eps_t = consts.tile([1, 1], f32, tag="eps_t")
            nc.vector.memset(eps_t, EPS)
            id2 = consts.tile([1, 1], f32, tag="id2")
            nc.vector.memset(id2, 1.0)
            id2b = consts.tile([1, 1], bf16, tag="id2b")
            nc.vector.memset(id2b, 1.0)

            const_dmas = []

            def col_load(name, dram, ntiles):
                t = consts.tile([P, ntiles], f32, tag=name)
                # DMA issued inside emit_block, after wv/ncs on the ACT queue
                const_dmas.append((t, dram.rearrange("(k p) -> p k", p=P)))
                return t

            bo_c = col_load("bo_c", bo_d, KT)
            b2_c = col_load("b2_c", b2_d, KT)
            b1_c = col_load("b1_c", b1_d, JT)

            for _ in range(repeat):
                emit_block(nc, tc, persist, wstream, w2pool, epool, scratch, rbpool,
                           rows, outp, ones_k, ones_kb, ones_r, ones_rb, eps_t, id2,
                           id2b, mask10, mask01, const_dmas, bo_c, b2_c, b1_c,
                           xt_d, xt8_d, wq_d, wv_d, cqv_d, wo_d, w1_d, w2_d, out_d,
                           upto)
    # NOTE: steering the ACT table chooser onto natural_log_exp (sqrt via
    # exp(0.5*ln(v))) cut table loads 8->4 but measured ~50us SLOWER on HW
    # (427/440us vs 387us twice) — the relocated loads evidently land at
    # worse points. Kept the plain Sqrt form.
    nc.compile()
    return nc


def emit_block(nc, tc, persist, wstream, w2pool, epool, scratch, rbpool, rows, outp,
               ones_k, ones_kb, ones_r, ones_rb, eps_t, id2, id2b, mask10, mask01,
               const_dmas, bo_c, b2_c, b1_c,
               xt_d, xt8_d, wq_d, wv_d, cqv_d, wo_d, w1_d, w2_d, out_d, upto="fc2"):
    TU = B2 * TE
    cu = lambda b: b * TE
    ct = lambda b: b * TP

    # Persistent tensors. Tags encode slot reuse; each second tenant's first
    # write is ordered after the first tenant's last read on every engine.
    xTs = [persist.tile([P, KT, TE], bf16, tag="sC", name="xT0"),
           persist.tile([P, KT, TE], bf16, tag="sD", name="xT1")]
    xT8s = [persist.tile([P, KT, TEP], fp8, tag="sX0", name="xT8_0"),
            persist.tile([P, KT, TEP], fp8, tag="sX1", name="xT8_1")]
    qT = persist.tile([P, KT, TU], bf16, tag="sE")
    vT = persist.tile([P, KT, B2 * TP], bf16, tag="sF")
    v_tm = persist.tile([P, B2, 5, VTP], fp8, tag="sG")
    wv_sb = persist.tile([P, KT, H], fp8, tag="sB")
    cqv_sb = persist.tile([P, 3, KT, P], bf16, tag="sQ")
    # mwide[b]: rows 0 = -mu, 1 = std, 2.. = 0 — the K=128 LN-correction
    # rhs folded into the q/v accumulations (q = rb_r * (Wq^T x - mu*csum + std*bq))
    mwides = [persist.tile([P, TE], bf16, tag="sM0", name="mw0"),
              persist.tile([P, TE], bf16, tag="sM1", name="mw1")]
    rtms = [rows.tile([P, 5, 2], f32, tag="rtm0", name="rtm0"),
            rows.tile([P, 5, 2], f32, tag="rtm1", name="rtm1")]
    rb_r_sbs = [None, None]

    def f(ap):
        return ap.bitcast(f32)

    def two(ap):
        return ap.rearrange("p (two c) -> p two c", two=2)

    # ---- load x (zero the 3 pad token columns) and wv ----
    for b in range(B2):
        for kt in range(KT):
            nc.sync.dma_start(xTs[b][:, kt, :T], xt_d[b, kt * P:(kt + 1) * P, :])
            nc.gpsimd.memset(xTs[b][:, kt, T:TE], 0.0)
            nc.sync.dma_start(xT8s[b][:, kt, :T], xt8_d[b, kt * P:(kt + 1) * P, :])
            nc.gpsimd.memset(xT8s[b][:, kt, T:TE], 0.0)
    nc.scalar.dma_start(wv_sb, wv_d.rearrange("(kt p) n -> p kt n", p=P))
    nc.scalar.dma_start(cqv_sb, cqv_d)
    for _t, _ap in const_dmas:
        nc.scalar.dma_start(_t, _ap)
    const_dmas.clear()
    for b in range(B2):
        for kt in range(KT):
            nc.gpsimd.memset(vT[:, kt, ct(b) + TE:ct(b) + TP], 0.0)

    # ---- LN1: stats only (no apply pass). Produces, per item:
    #  mwides[b] rows (-mu, std) for the in-psum correction matmul,
    #  rb_r_sbs[b] = broadcast 1/std for the q/v epilogue multiply,
    #  rtms[b] r-columns (token-major rstd) for the v_tm epilogue. ----
    def ln1_stats(b, ps):
        src = xTs[b]
        s1 = ps.tile([1, 1024], f32, tag="p", name="ln_s1")
        for kt in range(KT):
            for i in range(2):
                nc.tensor.matmul(s1[:, i * 512:i * 512 + HB], ones_kb,
                                 src[:, kt, i * HB:(i + 1) * HB],
                                 start=kt == 0, stop=kt == KT - 1)
        s2 = ps.tile([1, 1024], f32, tag="p", name="ln_s2")
        for kt in range(KT):
            sq = scratch.tile([P, TE], bf16, tag="sq")
            nc.vector.tensor_tensor(sq, src[:, kt, :], src[:, kt, :], OP.mult)
            for i in range(2):
                nc.tensor.matmul(s2[:, i * 512:i * 512 + HB], ones_kb,
                                 sq[:, i * HB:(i + 1) * HB],
                                 start=kt == 0, stop=kt == KT - 1)
        mu = rows.tile([1, TE], f32, tag="mu")
        nc.vector.tensor_scalar_mul(two(mu), halves(s1, 1), 1.0 / H)
        m2 = rows.tile([1, TE], f32, tag="var")
        nc.vector.tensor_tensor(m2, mu, mu, OP.mult)
        std = rows.tile([1, TE], f32, tag="std")
        nc.vector.tensor_scalar(two(std), halves(s2, 1), 1.0 / H, EPS,
                                OP.mult, OP.add)
        nc.vector.tensor_tensor(std, std, m2, OP.subtract)
        nc.scalar.activation(std, std, AF.Sqrt)
        nc.vector.tensor_copy(mwides[b][0:1, :], std)
        # -mu lives on partition 32: engine APs may only start at 0/32/64/96
        nc.vector.tensor_scalar_mul(mwides[b][32:33, :], mu, -1.0)
        rbp = ps.tile([P, 1024], f32, tag="p", name="ln_rbp")
        for i in range(2):
            nc.tensor.matmul(rbp[:, i * 512:i * 512 + HB], ones_rb,
                             mwides[b][0:1, i * HB:(i + 1) * HB],
                             start=True, stop=True)
        rb_r_sb = rbpool.tile([P, TE], bf16, tag="rbr", name="rb_r_sb")
        with nc.allow_low_precision(reason="bf16 rounding of rstd broadcast"):
            nc.vector.reciprocal(two(rb_r_sb), halves(rbp))
        rb_r_sbs[b] = rb_r_sb
        for ti in range(5):
            t0, tl = TT[ti]
            w = min(TE - t0, P)
            ptr = ps.tile([P, 1024], f32, tag="p", name="tr_ps")
            nc.tensor.transpose(ptr.bitcast(bf16)[:w, 0:1],
                                mwides[b][0:1, t0:t0 + w], id2b)
            with nc.allow_low_precision(reason="f32 recip of std column"):
                nc.vector.reciprocal(rtms[b][:w, ti, 0:1],
                                     ptr.bitcast(bf16)[:w, 0:1])

    # ---- layernorm with apply (used for LN2 only) ----
    def layernorm_b(src, s0, dst, d0, ps):
        bfsrc = src.dtype == bf16
        sf = (lambda ap: ap) if bfsrc else f
        ones_s1 = ones_kb if bfsrc else ones_k
        sfull = lambda kt: src[:, kt, s0:s0 + TE]
        shalf = lambda kt, i: src[:, kt, s0 + i * HB:s0 + (i + 1) * HB]
        s1 = ps.tile([1, 1024], f32, tag="p", name="ln_s1")
        for kt in range(KT):
            for i in range(2):
                nc.tensor.matmul(s1[:, i * 512:i * 512 + HB], ones_s1, shalf(kt, i),
                                 start=kt == 0, stop=kt == KT - 1)
        s2 = ps.tile([1, 1024], f32, tag="p", name="ln_s2")
        for kt in range(KT):
            sq = scratch.tile([P, TE], bf16, tag="sq")
            nc.vector.tensor_tensor(sq, sf(sfull(kt)), sf(sfull(kt)), OP.mult)
            for i in range(2):
                nc.tensor.matmul(s2[:, i * 512:i * 512 + HB], ones_kb,
                                 sq[:, i * HB:(i + 1) * HB],
                                 start=kt == 0, stop=kt == KT - 1)
        mu = rows.tile([1, TE], f32, tag="mu")
        nc.vector.tensor_scalar_mul(two(mu), halves(s1, 1), 1.0 / H)
        var = rows.tile([1, TE], f32, tag="var")
        nc.vector.tensor_tensor(var, mu, mu, OP.mult)
        nc.vector.scalar_tensor_tensor(two(var), halves(s2, 1), 1.0 / H,
                                       two(var), OP.mult, OP.subtract)
        nc.scalar.activation(var, var, AF.Sqrt, bias=eps_t)
        rstd = rows.tile([1, TE], f32r, tag="rstd")
        with nc.allow_low_precision(reason="f32r rounding of rstd"):
            nc.vector.reciprocal(rstd, var)
        murstd = rows.tile([1, TE], f32r, tag="var")
        nc.vector.tensor_tensor(murstd, mu, f(rstd), OP.mult)
        rb_r = ps.tile([P, 1024], f32, tag="p", name="ln_rbr")
        rb_mr = ps.tile([P, 1024], f32, tag="p", name="ln_rbmr")
        for i in range(2):
            nc.tensor.matmul(rb_r[:, i * 512:i * 512 + HB], ones_r,
                             rstd[:, i * HB:(i + 1) * HB], start=True, stop=True)
            nc.tensor.matmul(rb_mr[:, i * 512:i * 512 + HB], ones_r,
                             murstd[:, i * HB:(i + 1) * HB], start=True, stop=True)
        rb_r_sb = rbpool.tile([P, TE], bf16, tag="rbr", name="rb_r_sb")
        nc.vector.tensor_copy(two(rb_r_sb), halves(rb_r))
        rb_mr_sb = rbpool.tile([P, TE], bf16, tag="rbmr", name="rb_mr_sb")
        nc.scalar.activation(two(rb_mr_sb), halves(rb_mr), AF.Identity)
        for kt in range(KT):
            tmp = scratch.tile([P, TE], bf16, tag="lntmp")
            nc.vector.tensor_tensor(tmp, sf(sfull(kt)), rb_r_sb, OP.mult)
            nc.vector.tensor_tensor(dst[:, kt, d0:d0 + TE], tmp,
                                    rb_mr_sb, OP.subtract)
        return rstd, murstd

    with (tc.tile_pool(name="ps_ln1", bufs=2, space="PSUM") as ps_ln,
          tc.tile_pool(name="ps_vtm", bufs=4, space="PSUM") as ps_vtm):
        # token-major v from xT directly (no LN dependency): the matmuls fill
        # the PE during the LN1 latency chain; the -mu*colsum(wv) correction
        # rides as an extra K=128 matmul (mwide rows x cqv slot 2), so the
        # epilogue is a single per-token r multiply:
        #   v_tm[t,c] = r[t]*((Wv^T x)[t,c] - mu[t]*colsum(wv)[c])
        vtm_ps = {}

        def vtm_mms(b, ti, nb, pool=None, ptag="pv", psz=512):
            t0, tl = TT[ti]
            w = min(TE - t0, P)
            pv = (pool or ps_vtm).tile([P, psz], f32, tag=ptag, name="vtm_ps")
            for kp in range(KT // 2):
                nc.tensor.matmul(
                    pv[:w, :384], xT8s[b][:, 2 * kp:2 * kp + 2, t0:t0 + w],
                    wv_sb[:, 2 * kp:2 * kp + 2, nb * 384:(nb + 1) * 384],
                    start=kp == 0, stop=False, perf_mode=DR)
            nc.tensor.matmul(
                pv[:w, :384], mwides[b][:, t0:t0 + w],
                cqv_sb[:, 2].rearrange("p kt n -> p (kt n)")[:, nb * 384:(nb + 1) * 384],
                start=False, stop=True)
            vtm_ps[(b, ti, nb)] = pv

        def vtm_epi(b, ti, nb):
            rr = 128 if ti < 4 else 65
            pv = vtm_ps.pop((b, ti, nb))
            r_tm = rtms[b][:rr, ti, 0:1]
            nc.vector.tensor_scalar(
                v_tm[:rr, b, ti, 65 * nb * 6: 65 * (nb + 1) * 6]
                    .rearrange("p (h d) -> p h d", d=65)[:, :, :HD],
                pv[:rr, :384].rearrange("p (h d) -> p h d", d=HD),
                r_tm, 1.0 / SA, OP.mult, OP.mult)

        for b in range(B2):
            nc.gpsimd.memset(mwides[b], 0.0)
            nc.gpsimd.memset(v_tm[:, b, 4, :], 0.0)
            for ti in range(5):
                rr = 128 if ti < 4 else 65
                nc.gpsimd.memset(v_tm[:rr, b, ti, HD::65], 1.0)
        ln1_stats(0, ps_ln)
        for ti in range(5):
            for nb in range(2):
                vtm_mms(0, ti, nb)
        ln1_stats(1, ps_ln)
        for ti in range(5):
            for nb in range(2):
                vtm_mms(1, ti, nb)
        for b in range(B2):
            for ti in range(5):
                for nb in range(2):
                    vtm_epi(b, ti, nb)
    if upto == "ln1":
        return

    # ---- q/v, attention, wo+LN2, MLP — emitted with cross-phase interleave:
    # qv(b1) units ride inside att(b0)'s head loop, fc1(b0, j0-11) units ride
    # inside att(b1)'s. Emission order drives both the scheduler priorities
    # and the PSUM tag-"p" slot-reuse chain, so PE fill work is available
    # whenever the softmax (ACT) pipeline lags. ----
    oT = persist.tile([P, KT, TUP], fp8, tag="sH")
    x1T = persist.tile([P, KT, TU], bf16, tag="sA")
    y2dt = fp8 if FP8_FC1 else bf16
    y2Ts = [persist.tile([P, KT, TE], y2dt, tag="sB", name="y2T0"),
            persist.tile([P, KT, TE], y2dt, tag="sC", name="y2T1")]
    mtbox = [None]  # fc1 output chunks; 2 fresh slots + 2 reused (sG/sH)

    with (tc.tile_pool(name="ps_mid", bufs=2, space="PSUM") as ps,
          tc.tile_pool(name="ps_sc", bufs=2, space="PSUM") as ps_sc):

        def q_unit(b, j):
            wqj = wstream.tile([P, KT, P], fp8, tag="w8", name="wq_j")
            nc.sync.dma_start(wqj, wq_d[j])
            pq = ps.tile([P, 1024], f32, tag="p", name="q_ps")
            for kp in range(KT // 2):
                for i in range(2):
                    nc.tensor.matmul(
                        pq[:, i * 512:i * 512 + HB], wqj[:, 2 * kp:2 * kp + 2, :],
                        xT8s[b][:, 2 * kp:2 * kp + 2, i * HB:(i + 1) * HB],
                        start=kp == 0, stop=False, perf_mode=DR)
            for i in range(2):
                nc.tensor.matmul(pq[:, i * 512:i * 512 + HB], cqv_sb[:, 0, j, :],
                                 mwides[b][:, i * HB:(i + 1) * HB],
                                 start=False, stop=True)
            nc.vector.tensor_tensor(two(qT[:, j, cu(b):cu(b) + TE]),
                                    halves(pq), two(rb_r_sbs[b]), OP.mult)

        def v_unit(b, j):
            pv = ps.tile([P, 1024], f32, tag="p", name="v_ps")
            for kp in range(KT // 2):
                for i in range(2):
                    nc.tensor.matmul(
                        pv[:, i * 512:i * 512 + HB],
                        wv_sb[:, 2 * kp:2 * kp + 2, j * P:(j + 1) * P],
                        xT8s[b][:, 2 * kp:2 * kp + 2, i * HB:(i + 1) * HB],
                        start=kp == 0, stop=False, perf_mode=DR)
            for i in range(2):
                nc.tensor.matmul(pv[:, i * 512:i * 512 + HB], cqv_sb[:, 1, j, :],
                                 mwides[b][:, i * HB:(i + 1) * HB],
                                 start=False, stop=True)
            nc.vector.tensor_tensor(two(vT[:, j, ct(b):ct(b) + TE]),
                                    halves(pv), two(rb_r_sbs[b]), OP.mult)

        def qv_units(b):
            us = [lambda j=j: q_unit(b, j) for j in range(KT)]
            us += [lambda j=j: v_unit(b, j) for j in range(KT)]
            return us

        def scores_pair(b, hp):
            # heads (2hp, 2hp+1) live on partitions 0-63 / 64-127 of the same
            # vT/qT tile, so their K=64 matmuls land on PE row-tiles T0/T8.
            # Emitting them adjacently lets the halves run concurrently.
            jh = hp
            E0 = epool.tile([P, 5, TEP], fp8, tag="E", name="E0")
            E1 = epool.tile([P, 5, TEP], fp8, tag="E", name="E1")
            for kt in range(5):
                ps0 = ps_sc.tile([P, 1024], f32, tag="pss", name="s_ps0")
                ps1 = ps_sc.tile([P, 1024], f32, tag="pss", name="s_ps1")
                for i in range(2):
                    for ph, pst in ((0, ps0), (HD, ps1)):
                        nc.tensor.matmul(
                            pst[:, i * 512:i * 512 + HB],
                            vT[ph:ph + HD, jh, ct(b) + kt * P: ct(b) + (kt + 1) * P],
                            qT[ph:ph + HD, jh, cu(b) + i * HB: cu(b) + (i + 1) * HB],
                            start=True, stop=True)
                nc.scalar.activation(two(E0[:, kt, :TE]), halves(ps0),
                                     AF.Exp, scale=0.125)
                nc.scalar.activation(two(E1[:, kt, :TE]), halves(ps1),
                                     AF.Exp, scale=0.125)
            return E0, E1

        def out_pair(b, h0, E0, E1):
            # pnt for both heads, then the reciprocal broadcasts packed into
            # one PSUM tile on column-tiles (0,0)/(0,64) so they overlap.
            pnts = []
            for h, E in ((h0, E0), (h0 + 1, E1)):
                pnt = ps.tile([P, 1024], f32, tag="p", name="nt_ps")
                for i in range(2):
                    for m in range(2):
                        nc.tensor.matmul(pnt[:65, i * 512:i * 512 + HB],
                                         v_tm[:, b, 2 * m:2 * m + 2, 65 * h: 65 * h + 65],
                                         E[:, 2 * m:2 * m + 2, i * HB:(i + 1) * HB],
                                         start=m == 0, stop=False, perf_mode=DR)
                    nc.tensor.matmul(pnt[:65, i * 512:i * 512 + HB],
                                     v_tm[:, b, 4, 65 * h: 65 * h + 65],
                                     E[:, 4, i * HB:(i + 1) * HB],
                                     start=False, stop=True)
                pnts.append(pnt)
            rs = []
            for k, pnt in enumerate(pnts):
                r = rows.tile([1, TE], f32r, tag=f"r{k}", name=f"r{k}")
                with nc.allow_low_precision(reason="f32r rounding of softmax denom"):
                    nc.vector.reciprocal(two(r), halves(pnt, 128)[64:65])
                rs.append(r)
            prr = ps_sc.tile([P, 1024], f32, tag="pss", name="rep_ps")
            for k, (r, msk) in enumerate(zip(rs, (mask10, mask01))):
                for i in range(2):
                    nc.tensor.matmul(prr[:, i * 512:i * 512 + HB], msk,
                                     r[:, i * HB:(i + 1) * HB],
                                     start=k == 0, stop=k == 1)
            prr2 = prr.rearrange("p (two c) -> p two c", two=2)
            for k, h in enumerate((h0, h0 + 1)):
                jh, ph = h // 2, HD * (h % 2)
                pnt = pnts[k]
                ntsb = scratch.tile([P, TE], f32, tag="ntsb")
                if k == 0:
                    nc.vector.tensor_copy(two(ntsb[:HD]), halves(pnt, HD))
                else:
                    # split the PSUM evacuations across DVE and ACT (Copy is
                    # in every ACT table set, so no table-switch cost)
                    nc.scalar.activation(two(ntsb[:HD]), halves(pnt, HD),
                                         AF.Copy)
                nc.vector.tensor_tensor(
                    two(oT[ph:ph + HD, jh, cu(b):cu(b) + TE]),
                    two(ntsb[:HD]), prr2[64 * k:64 * k + HD, :, :HB], OP.mult)

        def attention_b(b, fill, per_head, defer_last=0):
            # scores run one pair ahead of out_pair so the exp (ACT) chain
            # never waits behind pnt/prr matmuls in the PE queue; the last
            # `defer_last` out_pairs can be returned as pending units to
            # thicken the next attention window's fill
            Es = {}
            NP = NH // 2
            pend = []

            def emit_out(hp):
                E0, E1 = Es.pop(hp)
                out_pair(b, 2 * hp, E0, E1)

            def place(hp):
                if hp >= NP - defer_last:
                    pend.append(lambda hp=hp: emit_out(hp))
                else:
                    emit_out(hp)

            for hp in range(NP):
                Es[hp] = scores_pair(b, hp)
                if hp >= 1:
                    place(hp - 1)
                for _ in range(2 * per_head):
                    if fill:
                        fill.pop(0)()
            place(NP - 1)
            while fill:
                fill.pop(0)()
            return pend

        def wo_unit(b, j):
            woj = wstream.tile([P, KT, P], fp8, tag="w8", name="wo_j")
            nc.sync.dma_start(woj, wo_d[j])
            po = ps.tile([P, 1024], f32, tag="p", name="wo_ps")
            for kp in range(KT // 2):
                for i in range(2):
                    nc.tensor.matmul(
                        po[:, i * 512:i * 512 + HB], woj[:, 2 * kp:2 * kp + 2, :],
                        oT[:, 2 * kp:2 * kp + 2, cu(b) + i * HB: cu(b) + (i + 1) * HB],
                        start=kp == 0, stop=kp == KT // 2 - 1, perf_mode=DR)
            # (po/SA + bo) on DVE, + residual on Pool (idle mid-attention)
            x1w = scratch.tile([P, TE], f32, tag="x1w")
            nc.vector.tensor_scalar(two(x1w), halves(po), 1.0 / SA,
                                    bo_c[:, j:j + 1], OP.mult, OP.add)
            nc.gpsimd.tensor_tensor(x1T[:, j, cu(b):cu(b) + TE], x1w,
                                    xTs[b][:, j, :], OP.add)

        def wo_ln2(b):
            for j in range(KT):
                wo_unit(b, j)
            layernorm_b(x1T, cu(b), y2Ts[b], 0, ps)

        def fc1_unit(bb, j, defer_gelu=False, pool=None, ptag="p"):
            w1j = wstream.tile([P, KT, P], fp8 if FP8_FC1 else bf16,
                               tag="w", name="w1_j")
            nc.sync.dma_start(w1j, w1_d[j])
            pm = (pool or ps).tile([P, 1024], f32, tag=ptag, name="fc1_ps")
            if FP8_FC1:
                for kp in range(KT // 2):
                    for i in range(2):
                        nc.tensor.matmul(
                            pm[:, i * 512:i * 512 + HB], w1j[:, 2 * kp:2 * kp + 2, :],
                            y2Ts[bb][:, 2 * kp:2 * kp + 2, i * HB:(i + 1) * HB],
                            start=kp == 0, stop=kp == KT // 2 - 1, perf_mode=DR)
            else:
                for kt in range(KT):
                    for i in range(2):
                        nc.tensor.matmul(
                            pm[:, i * 512:i * 512 + HB], w1j[:, kt, :],
                            y2Ts[bb][:, kt, i * HB:(i + 1) * HB],
                            start=kt == 0, stop=kt == KT - 1)
            sc1 = (1.0 / S1) if FP8_FC1 else 1.0
            dst = two(mtbox[0][j // KT][:, j % KT, cu(bb):cu(bb) + TE])
            if defer_gelu:
                # raw+bias on DVE (ACT is exp-saturated mid-attention);
                # gelu applied in place later, gated behind attention's end
                nc.vector.tensor_scalar(dst, halves(pm), sc1,
                                        b1_c[:, j:j + 1], OP.mult, OP.add)
            else:
                nc.scalar.activation(dst, halves(pm), AF.Gelu,
                                     bias=b1_c[:, j:j + 1], scale=sc1)

        def fc2_chain(bb, j2, w2j):
            pf = (ps_sc if (j2 + bb) % 2 else ps).tile(
                [P, 1024], f32, tag="pss" if (j2 + bb) % 2 else "p",
                name="fc2_ps")
            if FP8_FC2:
                for jp in range(JT // 2):
                    for i in range(2):
                        nc.tensor.matmul(
                            pf[:, i * 512:i * 512 + HB], w2j[:, 2 * jp:2 * jp + 2, :],
                            mtbox[0][(2 * jp) // KT][:, (2 * jp) % KT:(2 * jp) % KT + 2,
                                                     cu(bb) + i * HB: cu(bb) + (i + 1) * HB],
                            start=jp == 0, stop=jp == JT // 2 - 1, perf_mode=DR)
                ob = outp.tile([P, TE], f32, tag="ob")
                nc.vector.tensor_scalar(two(ob), halves(pf), 1.0 / S2,
                                        b2_c[:, j2:j2 + 1], OP.mult, OP.add)
                nc.gpsimd.tensor_tensor(ob, ob,
                                        x1T[:, j2, cu(bb):cu(bb) + TE], OP.add)
            else:
                for j in range(JT):
                    for i in range(2):
                        nc.tensor.matmul(
                            pf[:, i * 512:i * 512 + HB], w2j[:, j, :],
                            mtbox[0][j // KT][:, j % KT, cu(bb) + i * HB: cu(bb) + (i + 1) * HB],
                            start=j == 0, stop=j == JT - 1)
                ob = outp.tile([P, TE], f32, tag="ob")
                nc.vector.scalar_tensor_tensor(
                    two(ob), halves(pf), b2_c[:, j2:j2 + 1],
                    two(x1T[:, j2, cu(bb):cu(bb) + TE]), OP.add, OP.add)
            nc.sync.dma_start(
                out_d[bb, j2 * P:(j2 + 1) * P, :], ob[:, :T])

        for u in qv_units(0):
            u()
        if upto == "qv":
            return
        pend0 = attention_b(0, qv_units(1), 2,
                            defer_last=0 if upto != "fc2" else 2)
        if upto in ("att", "wo", "ln2"):
            wo_ln2(0)
            return
        mtbox[0] = [persist.tile([P, KT, TU], fp8 if FP8_FC2 else bf16,
                                 tag=t, name=f"mt_{t}")
                    for t in ("sI", "sJ", "sG", "sH")]
        # att(b0)'s last two out_pairs lead the fill: att(b1)'s early pairs
        # are exp-paced with little other PE work available
        fill1 = pend0 + [lambda j=j: wo_unit(0, j) for j in range(KT)]
        fill1.append(lambda: layernorm_b(x1T, cu(0), y2Ts[0], 0, ps))
        fill1 += [lambda j=j: fc1_unit(0, j, defer_gelu=True)
                  for j in range(2 * KT)]
        attention_b(1, fill1, 2)
        # wo(1) interleaved with the remaining fc1(0) units (also gelu-
        # deferred) so the in-order PE stream has fill while LN2(b1)'s
        # latency chain runs.
        for j in range(KT):
            wo_unit(1, j)
            fc1_unit(0, 2 * KT + 2 * j, defer_gelu=True, pool=ps_sc, ptag="pss")
            fc1_unit(0, 2 * KT + 2 * j + 1, defer_gelu=True, pool=ps_sc,
                     ptag="pss")
        layernorm_b(x1T, cu(1), y2Ts[1], 0, ps)
        # zero bias gate, data-dependent on the last out_head of att(b1):
        # keeps the deferred gelus (a different ACT table) from hoisting into
        # the attention window and thrashing LoadActFuncSet.
        gate0 = rows.tile([P, 1], f32, tag="gate")
        nc.gpsimd.tensor_scalar(gate0, f(oT.bitcast(f32)[:, KT - 1, (cu(1) + TE) // 4 - 1: (cu(1) + TE) // 4]),
                                0.0, None, OP.mult)
        for j in range(JT):
            sl = mtbox[0][j // KT][:, j % KT, cu(0):cu(0) + TE]
            nc.scalar.activation(sl, sl, AF.Gelu, bias=gate0)
        for j in range(JT):
            fc1_unit(1, j, pool=ps_sc, ptag="pss")
        if upto == "fc1":
            return
        for j2 in range(KT):
            w2j = w2pool.tile([P, JT, P], fp8 if FP8_FC2 else bf16,
                              tag="w2", name="w2_j")
            nc.sync.dma_start(w2j, w2_d[j2])
            for bb in range(B2):
                fc2_chain(bb, j2, w2j)


_cached = {}


def get_program(repeat=1):
    if repeat not in _cached:
        _cached[repeat] = build_program(repeat)
    return _cached[repeat]


def make_in_maps(inputs):
    x = np.asarray(inputs["x"], dtype=np.float32)
    xt_all = np.ascontiguousarray(x.transpose(0, 2, 1))  # [B, H, T]
    g1 = np.asarray(inputs["ln1_g"], np.float64)
    be1 = np.asarray(inputs["ln1_b"], np.float64)
    g2 = np.asarray(inputs["ln2_g"], np.float64)
    be2 = np.asarray(inputs["ln2_b"], np.float64)
    Wq = np.asarray(inputs["Wq"], np.float64)
    Wv = np.asarray(inputs["Wv"], np.float64)
    Wo = np.asarray(inputs["Wo"], np.float64)
    W1 = np.asarray(inputs["W1"], np.float64)
    # Fold LN affine into the consuming projections (exact refactoring).
    wq = g1[:, None] * Wq
    bq = be1 @ Wq + np.asarray(inputs["bq"], np.float64)
    wv = g1[:, None] * Wv
    bv = be1 @ Wv + np.asarray(inputs["bv"], np.float64)
    # v_tm carries no bias; probs rows sum to 1 so P@(1 x bv) == bv -> fold into bo.
    bo = np.asarray(inputs["bo"], np.float64) + bv @ Wo
    w1 = g2[:, None] * W1
    b1 = be2 @ W1 + np.asarray(inputs["b1"], np.float64)
    def prep(w, jt, dt):
        # [H_in, J*128] -> [j, p, kt, n] so each per-j DMA is fully contiguous
        kt = w.shape[0] // P
        return np.ascontiguousarray(
            w.reshape(kt, P, jt, P).transpose(2, 1, 0, 3)).astype(dt)

    # fp8 attention-path weights, scaled by SA; column sums for the LN
    # correction are taken over the QUANTIZED weights so the mean term
    # cancels exactly.
    wq8 = (wq * SA).astype(ml_dtypes.float8_e4m3fn)
    wv8 = (wv * SA).astype(ml_dtypes.float8_e4m3fn)
    wo8 = (np.asarray(inputs["Wo"], np.float64) * SA).astype(ml_dtypes.float8_e4m3fn)
    # LN-correction lhsT consts, paired against mwide rows (0: std, 32: -mu):
    # slot 0 = q (SA*bq, csum(wq8)), slot 1 = v (SA*bv, csum(wv8)),
    # slot 2 = v_tm (0, csum(wv8)); all other rows zero.
    cqv = np.zeros((P, 3, KT, P), np.float64)
    cqv[0, 0] = (SA * bq).reshape(KT, P)
    cqv[32, 0] = wq8.astype(np.float64).sum(axis=0).reshape(KT, P)
    cqv[0, 1] = (SA * bv).reshape(KT, P)
    cqv[32, 1] = wv8.astype(np.float64).sum(axis=0).reshape(KT, P)
    cqv[32, 2] = wv8.astype(np.float64).sum(axis=0).reshape(KT, P)

    def prep8(w8, jt):
        kt = w8.shape[0] // P
        return np.ascontiguousarray(
            w8.reshape(kt, P, jt, P).transpose(2, 1, 0, 3))

    com = {
        "wq": prep8(wq8, KT),
        "cqv": cqv.astype(ml_dtypes.bfloat16),
        "wv": wv8,
        "wo": prep8(wo8, KT),
        "w1": prep(w1 * S1, JT, ml_dtypes.float8_e4m3fn) if FP8_FC1
        else prep(w1, JT, ml_dtypes.bfloat16),
        "w2": prep(np.asarray(inputs["W2"], np.float64) * S2, KT,
                   ml_dtypes.float8_e4m3fn) if FP8_FC2
        else prep(np.asarray(inputs["W2"], np.float64), KT, ml_dtypes.bfloat16),
        "bo": bo.astype(np.float32),
        "b1": b1.astype(np.float32),
        "b2": np.asarray(inputs["b2"], np.float32),
    }
    return [dict(com,
                 xt=np.ascontiguousarray(xt_all[i * B2:(i + 1) * B2]).astype(ml_dtypes.bfloat16),
                 xt8=np.ascontiguousarray(xt_all[i * B2:(i + 1) * B2]).astype(ml_dtypes.float8_e4m3fn))
            for i in range(NCORES)]


def kernel(**inputs):
    nc = get_program()
    in_maps = make_in_maps(inputs)
    res = bass_utils.run_bass_kernel_spmd(nc, in_maps, core_ids=list(range(NCORES)))
    out = np.concatenate([res.results[i]["outt"] for i in range(NCORES)], axis=0)
    return np.ascontiguousarray(out.transpose(0, 2, 1)).astype(np.float32)

